# revision 1
# baseline (speedup 1.0000x reference)
"""Trainium2 Bass kernel v2 for nn_AffineContour — collective-free.

Design (8 cores, one trn2 chip):
- Even devices (even pids) compute the u-MLP, odd devices the v-MLP;
  each group of 4 row-shards its (65536, 512) W0 4 ways (8.4 MB fp8
  per core, 4x less HBM traffic than the fp32 8-way baseline).
- W0 host-quantized to fp8e4 per-column; x_even fp8 with one global
  scale; scales fold into W1 rows (relu(s z) = s relu(z), s>0); W1 bf16.
- Intra-group exchange of the [128,4] layer-1 partial runs on
  remote_dma_broadcast with relative dtpb {2,4,6} (parity-preserving —
  odd dtpb is unroutable on this fabric). No ncfw collectives.
- The final u_s/u_t scalars cross the parity boundary through the
  pair-shared HBM scratchpad (cores 2k,2k+1 share an HBM domain) with a
  nonce-flagged write + gpsimd poll loop.
- Compute that needs PSUM lives in tile regions (raw PSUM addressing is
  broken); raw regions only do SBUF DMA, remote descgen and sem work.
"""

import os
import threading

import ml_dtypes
import numpy as np

import concourse.bacc as bacc
import concourse.bass as bass
import concourse.mybir as mybir
import concourse.tile as tile
from concourse.bass_utils import run_bass_kernel_spmd

V = 131072
VH = 65536
F = 512
NCORES = 8
CHUNKS = 128          # 128-row chunks per core (16384 rows)
G = 8                 # chunks per weight DMA group
NG = CHUNKS // G      # 16 groups per core
F32 = mybir.dt.float32
BF16 = mybir.dt.bfloat16
FP8 = mybir.dt.float8e4
I32 = mybir.dt.int32

NP_FP8 = ml_dtypes.float8_e4m3   # IEEE e4m3, max 240 — matches TRN fp8e4
NP_BF16 = ml_dtypes.bfloat16

_lock = threading.Lock()
_cache = {}


def build_nc():
    nc = bacc.Bacc(
        "TRN2",
        debug=False,
        enable_asserts=False,
        target_bir_lowering=False,
        num_devices=NCORES,
    )
    DEBUG = os.environ.get("KV2_DEBUG", "0") == "1"

    w0 = nc.dram_tensor("w0", [NG, 128, G * F], FP8, kind="ExternalInput")
    xe8_d = nc.dram_tensor("xe8", [128, CHUNKS], FP8, kind="ExternalInput")
    w1_d = nc.dram_tensor("w1", [2, 128, 4 * F], BF16, kind="ExternalInput")
    b0s_d = nc.dram_tensor("b0s", [128, 8], F32, kind="ExternalInput")
    b1_d = nc.dram_tensor("b1", [1, 2 * F], BF16, kind="ExternalInput")
    hw_d = nc.dram_tensor("hw", [1, 2 * F], F32, kind="ExternalInput")
    hb_d = nc.dram_tensor("hb", [1, 2], F32, kind="ExternalInput")
    xo_d = nc.dram_tensor("xo_t", [128, 64], F32, kind="ExternalInput")
    nonce_d = nc.dram_tensor("nonce", [1, 1], F32, kind="ExternalInput")
    isodd_d = nc.dram_tensor("isodd", [1, 1], I32, kind="ExternalInput")

    vals_out = nc.dram_tensor("vals_out", [VH // NCORES], F32, kind="ExternalOutput")
    if DEBUG:
        dbg_recv = nc.dram_tensor("dbg_recv", [128, 16], F32, kind="ExternalOutput")
        dbg_poll = nc.dram_tensor("dbg_poll", [1, 4], F32, kind="ExternalOutput")
        dbg_wr = nc.dram_tensor("dbg_wr", [1, 2], F32, kind="ExternalOutput")

    shflag = nc.dram_tensor("shflag", [1, 4], F32, addr_space="Shared")
    shbig = nc.dram_tensor("shbig", [128, 32], F32, addr_space="Shared")

    # raw (persistent) SBUF shared across tile regions
    partial = nc.alloc_sbuf_tensor("partial_sb", [1, F], F32)
    zt = nc.alloc_sbuf_tensor("zt_sb", [128, 4], F32)
    recv = nc.alloc_sbuf_tensor("recv_sb", [128, 16], F32)
    w1_sb = nc.alloc_sbuf_tensor("w1_sb", [128, 2 * 4 * F], BF16)
    b0s = nc.alloc_sbuf_tensor("b0s_sb", [128, 8], F32)
    b1_sb = nc.alloc_sbuf_tensor("b1_sb", [1, 2 * F], BF16)
    hw_sb = nc.alloc_sbuf_tensor("hw_sb", [1, 2 * F], F32)
    hb_sb = nc.alloc_sbuf_tensor("hb_sb", [1, 2], F32)
    xo = nc.alloc_sbuf_tensor("xo_sb", [128, 64], F32)
    wr = nc.alloc_sbuf_tensor("wr_sb", [1, 2], F32)      # (scalar, nonce)
    pollb = nc.alloc_sbuf_tensor("poll_sb", [1, 4], F32)
    isodd_sb = nc.alloc_sbuf_tensor("isodd_sb", [1, 1], I32)
    one_f = nc.alloc_sbuf_tensor("one_f", [1, 1], F32)
    ones128 = nc.alloc_sbuf_tensor("ones128", [1, 128], F32)
    one_b = nc.alloc_sbuf_tensor("one_b", [1, 1], BF16)
    uvr = nc.alloc_sbuf_tensor("uvr_sb", [128, 8], BF16)
    acc = nc.alloc_sbuf_tensor("acc_sb", [128, 8], F32)
    junk = nc.alloc_sbuf_tensor("junk_sb", [1, F], F32)
    st = nc.alloc_sbuf_tensor("st_sb", [1, 2], F32)
    shcopy = nc.alloc_sbuf_tensor("shcopy_sb", [128, 32], F32)
    sc2 = nc.alloc_sbuf_tensor("sc2_sb", [1, 2], F32)
    st_T = nc.alloc_sbuf_tensor("stT_sb", [128, 2], F32)
    vals = nc.alloc_sbuf_tensor("vals_sb", [128, 64], F32)

    # semaphores (raw regions)
    rsem = nc.alloc_semaphore("rdma_rsem")
    lsem = nc.alloc_semaphore("rdma_lsem")
    psem = nc.alloc_semaphore("prep_sem")
    ldsem = nc.alloc_semaphore("load_sem")
    s_v2 = nc.alloc_semaphore("s_v2")
    s_v4 = nc.alloc_semaphore("s_v4")
    s_g1 = nc.alloc_semaphore("s_g1")
    wsem = nc.alloc_semaphore("shwr_sem")

    gp = nc.gpsimd

    # --- pre-tile raw: clear cross-run sems, start small loads ---
    for s in (rsem, lsem, psem, ldsem, s_v2, s_v4, s_g1, wsem):
        gp.sem_clear(s)
    gp.dma_start(w1_sb[:, 0 : 4 * F], w1_d.ap()[0]).then_inc(ldsem, 16)
    gp.dma_start(w1_sb[:, 4 * F : 8 * F], w1_d.ap()[1]).then_inc(ldsem, 16)
    gp.dma_start(b0s[:], b0s_d[:]).then_inc(ldsem, 16)
    gp.dma_start(b1_sb[:], b1_d[:]).then_inc(ldsem, 16)
    gp.dma_start(hw_sb[:], hw_d[:]).then_inc(ldsem, 16)
    gp.dma_start(hb_sb[:], hb_d[:]).then_inc(ldsem, 16)
    gp.dma_start(xo[:], xo_d[:]).then_inc(ldsem, 16)
    gp.dma_start(wr[0:1, 1:2], nonce_d[:]).then_inc(ldsem, 16)
    gp.dma_start(isodd_sb[:], isodd_d[:]).then_inc(ldsem, 16)
    NLOADS = 9

    # --- tile A: fp8 GEMV + transpose to zt [128,4] ---
    with tile.TileContext(nc) as tc:
        with (
            tc.tile_pool(name="wpool", bufs=5) as wpool,
            tc.tile_pool(name="spool", bufs=1) as spool,
            tc.tile_pool(name="psum", bufs=1, space="PSUM") as psum,
        ):
            xe = spool.tile([128, CHUNKS], FP8)
            nc.sync.dma_start(xe[:], xe8_d[:])
            nc.vector.memset(one_f[:], 1.0)
            nc.vector.memset(ones128[:], 1.0)
            nc.vector.memset(one_b[:], 1.0)
            nc.vector.memset(pollb[:], 0.0)

            dma_engines = [nc.sync, nc.scalar]
            psum1 = psum.tile([1, F], F32, name="psum1")
            for g in range(NG):
                wt = wpool.tile([128, G * F], FP8)
                if g == NG - 1:
                    for q in range(4):
                        dma_engines[g % 2].dma_start(
                            wt[:, 1024 * q : 1024 * (q + 1)],
                            w0.ap()[g][:, 1024 * q : 1024 * (q + 1)],
                        )
                else:
                    dma_engines[g % 2].dma_start(wt[:], w0.ap()[g])
                for t in range(G):
                    c = g * G + t
                    nc.tensor.matmul(
                        psum1[:],
                        xe[:, c : c + 1],
                        wt[:, t * F : (t + 1) * F],
                        start=(c == 0),
                        stop=(c == CHUNKS - 1),
                    )
            nc.vector.tensor_copy(partial[:], psum1[:])
            psum_t = psum.tile([128, 4], F32, name="psum_t")
            for r in range(4):
                nc.tensor.matmul(
                    psum_t[:, r : r + 1],
                    partial[0:1, 128 * r : 128 * (r + 1)],
                    one_f[0:1, 0:1],
                    start=True,
                    stop=True,
                )
            nc.vector.tensor_copy(zt[:], psum_t[:])
            nc.vector.tensor_copy(recv[:, 0:4], psum_t[:])

    # --- raw 1: intra-group exchange (dtpb 2/4/6 -> recv slots 1..3) ---
    nc.vector.sem_inc(s_v2, 1)
    gp.wait_ge(ldsem, 16 * NLOADS)
    gp.wait_ge(s_v2, 1)
    for i, (dtpb, rslot) in enumerate([(2, 2), (4, 4), (6, 6)]):
        rd: list = [None] * 8
        rd[rslot] = (0, dtpb)
        gp.remote_dma_broadcast(
            recv[:, 4 * (i + 1) : 4 * (i + 2)],
            zt[:],
            remote_sem=rsem,
            local_sem=lsem,
            rdests=rd,
        ).then_inc(psem, 1)
    gp.wait_ge(psem, 3)
    gp.trigger_dma(count=3)
    gp.wait_ge(rsem, 6)

    # buddy swap through pair-shared HBM: payload, then nonce flag
    with gp.register() as r_odd:
        gp.reg_load(r_odd, isodd_sb[0:1, 0:1])
        with gp.If(r_odd):
            gp.dma_start(shbig.ap()[:, 16:32], recv[:]).then_inc(wsem, 16)
            gp.wait_ge(wsem, 16)
            gp.dma_start(shflag.ap()[0:1, 2:4], wr[:]).then_inc(wsem, 16)
        with gp.Else():
            gp.dma_start(shbig.ap()[:, 0:16], recv[:]).then_inc(wsem, 16)
            gp.wait_ge(wsem, 16)
            gp.dma_start(shflag.ap()[0:1, 0:2], wr[:]).then_inc(wsem, 16)
    ms = nc.monotonic_semaphore(0)
    with gp.register() as r_n, gp.register() as r_e, gp.register() as r_o, \
            gp.register() as r_t, gp.register() as r_i:
        gp.reg_load(r_n, wr[0:1, 1:2].bitcast(I32))
        gp.reg_mov(r_i, 0)

        def cond():
            gp.reg_load(r_e, pollb[0:1, 1:2].bitcast(I32))
            gp.reg_load(r_o, pollb[0:1, 3:4].bitcast(I32))
            gp.reg_alu(r_e, r_e, r_n, mybir.AluOpType.bitwise_xor)
            gp.reg_alu(r_o, r_o, r_n, mybir.AluOpType.bitwise_xor)
            gp.reg_alu(r_t, r_e, r_o, mybir.AluOpType.bitwise_or)
            if DEBUG:
                gp.reg_alu(r_i, r_i, 1, mybir.AluOpType.add)
                with gp.register() as r_lt:
                    gp.reg_alu(r_lt, r_i, 64, mybir.AluOpType.is_lt)
                    gp.reg_alu(r_t, r_t, r_lt, mybir.AluOpType.logical_and)
            return r_t

        with gp.While(cond):
            gp.dma_start(pollb[:], shflag.ap()).then_inc(ms.sem(), 16)
            ms.wait_inc(16)
    gp.dma_start(shcopy[:], shbig.ap()).then_inc(ms.sem(), 16)
    ms.wait_inc(16)
    gp.sem_inc(s_g1, 1)
    nc.vector.wait_ge(s_g1, 1)
    nc.tensor.wait_ge(s_g1, 1)

    # --- tile B: both groups: reduce + relu + layer 2 + heads + vals ---
    with tile.TileContext(nc) as tc2:
        with (
            tc2.tile_pool(name="spool2", bufs=1) as spool2,
            tc2.tile_pool(name="psum2p", bufs=1, space="PSUM") as psum2p,
        ):
            for m in range(2):
                sl = shcopy[:, 16 * m : 16 * m + 16]
                am = acc[:, 4 * m : 4 * (m + 1)]
                nc.vector.tensor_tensor(am, sl[:, 0:4], sl[:, 4:8],
                                        op=mybir.AluOpType.add)
                nc.vector.tensor_tensor(am, am, sl[:, 8:12],
                                        op=mybir.AluOpType.add)
                nc.vector.tensor_tensor(am, am, sl[:, 12:16],
                                        op=mybir.AluOpType.add)
            nc.vector.tensor_tensor(acc[:], acc[:], b0s[:],
                                    op=mybir.AluOpType.add)
            nc.vector.tensor_relu(uvr[:], acc[:])
            for m in range(2):
                psum2 = psum2p.tile([1, F], F32, name=f"psum2_{m}")
                nc.tensor.matmul(psum2[:], one_b[0:1, :],
                                 b1_sb[:, m * F : (m + 1) * F],
                                 start=True, stop=False)
                for r in range(4):
                    nc.tensor.matmul(
                        psum2[:],
                        uvr[:, 4 * m + r : 4 * m + r + 1],
                        w1_sb[:, (4 * m + r) * F : (4 * m + r + 1) * F],
                        start=False,
                        stop=(r == 3),
                    )
                nc.vector.scalar_tensor_tensor(
                    junk[:], psum2[:], 0.0, hw_sb[:, m * F : (m + 1) * F],
                    op0=mybir.AluOpType.max, op1=mybir.AluOpType.mult,
                    accum_out=st[:, m : m + 1],
                )
            nc.vector.tensor_tensor(sc2[:], st[:], hb_sb[:],
                                    op=mybir.AluOpType.add)
            psum_bc = psum2p.tile([128, 2], F32, name="psum_bc")
            nc.tensor.matmul(psum_bc[:], ones128[:], sc2[:],
                             start=True, stop=True)
            nc.vector.tensor_copy(st_T[:], psum_bc[:])
            nc.vector.tensor_scalar(
                vals[:], xo[:], st_T[:, 0:1], st_T[:, 1:2],
                op0=mybir.AluOpType.mult, op1=mybir.AluOpType.add,
            )
            nc.sync.dma_start(
                vals_out.ap().rearrange("(p t) -> p t", p=128), vals[:]
            )
            if DEBUG:
                nc.sync.dma_start(dbg_recv.ap(), recv[:])
                nc.sync.dma_start(dbg_poll.ap(), pollb[:])
                nc.sync.dma_start(dbg_wr.ap(), wr[:])

    nc.compile()
    return nc


def _quant_w0(W0):
    """Per-column absmax fp8 quantization. Returns (W0q fp8, s per-col)."""
    W0 = np.asarray(W0, dtype=np.float32)
    s = np.abs(W0).max(axis=0).astype(np.float64) / 240.0
    s = np.where(s == 0, 1.0, s)
    W0q = (W0 / s).astype(NP_FP8)
    return W0q, s


def _pack_w0(W0q, q):
    # [65536, 512] -> this quarter's [NG, 128, G*F]
    A = W0q.reshape(128, 512, F)[:, 128 * q : 128 * (q + 1), :]
    A = A.reshape(128, NG, G, F).transpose(1, 0, 2, 3)
    return np.ascontiguousarray(A).reshape(NG, 128, G * F)


def _pack_w1(W1e):
    # w1p[p, r*F+n] = W1e[128r+p, n]
    return np.ascontiguousarray(
        W1e.reshape(4, 128, F).transpose(1, 0, 2)
    ).reshape(128, 4 * F)


def make_in_maps(
    x, u_W0, u_b0, u_W1, u_b1, v_W0, v_b0, v_W1, v_b1,
    us_W, us_b, ut_W, ut_b, even_indices, odd_indices,
):
    x = np.asarray(x, dtype=np.float32)
    xe = x[np.asarray(even_indices)].astype(np.float32)
    xo = x[np.asarray(odd_indices)].astype(np.float32)
    xe_m = xe.reshape(128, 512)

    sx = 240.0 / max(np.abs(xe).max(), 1e-30)
    xe_q = (xe_m * sx).astype(NP_FP8)

    nonce = np.float32(np.random.uniform(1.0, 2.0) * 1e30)

    groups = {}
    for gname, W0, b0, W1, b1, hW, hb in [
        ("u", u_W0, u_b0, u_W1, u_b1, us_W, us_b),
        ("v", v_W0, v_b0, v_W1, v_b1, ut_W, ut_b),
    ]:
        W0q, s = _quant_w0(W0)
        se = s / sx
        W1e = np.asarray(W1, np.float64) * se[:, None]
        b0se = (np.asarray(b0, np.float64) / se).astype(np.float32)
        groups[gname] = dict(
            W0q=W0q,
            w1=_pack_w1(W1e.astype(NP_BF16)),
            b0s=np.ascontiguousarray(b0se.reshape(4, 128).T),
            b1=np.asarray(b1, np.float32).astype(NP_BF16)[None, :],
            hw=np.asarray(hW, np.float32)[:, 0][None, :],
            hb=np.asarray(hb, np.float32)[None, :],
        )

    w1b = np.stack([groups["u"]["w1"], groups["v"]["w1"]])
    b0sb = np.concatenate([groups["u"]["b0s"], groups["v"]["b0s"]], axis=1)
    b1b = np.concatenate([groups["u"]["b1"], groups["v"]["b1"]], axis=1)
    hwb = np.concatenate([groups["u"]["hw"], groups["v"]["hw"]], axis=1)
    hbb = np.concatenate([groups["u"]["hb"], groups["v"]["hb"]], axis=1)

    rpc = VH // NCORES
    in_maps = []
    for j in range(NCORES):
        gname = "u" if j % 2 == 0 else "v"
        q = j // 2
        gr = groups[gname]
        in_maps.append(
            {
                "w0": _pack_w0(gr["W0q"], q),
                "xe8": np.ascontiguousarray(xe_q[:, 128 * q : 128 * (q + 1)]),
                "w1": w1b,
                "b0s": b0sb,
                "b1": b1b,
                "hw": hwb,
                "hb": hbb,
                "xo_t": xo[rpc * j : rpc * (j + 1)].reshape(128, 64),
                "nonce": np.full((1, 1), nonce, dtype=np.float32),
                "isodd": np.full((1, 1), j % 2, dtype=np.int32),
            }
        )
    return in_maps


def kernel(
    x, u_W0, u_b0, u_W1, u_b1, v_W0, v_b0, v_W1, v_b1,
    us_W, us_b, ut_W, ut_b, even_indices, odd_indices,
):
    x = np.asarray(x, dtype=np.float32)
    odd_indices = np.asarray(odd_indices)

    with _lock:
        if "nc" not in _cache:
            _cache["nc"] = build_nc()
    nc = _cache["nc"]

    in_maps = make_in_maps(
        x, u_W0, u_b0, u_W1, u_b1, v_W0, v_b0, v_W1, v_b1,
        us_W, us_b, ut_W, ut_b, even_indices, odd_indices,
    )

    res = run_bass_kernel_spmd(nc, in_maps, core_ids=list(range(NCORES)))
    vals = np.concatenate([res.results[j]["vals_out"] for j in range(NCORES)])

    imag = np.zeros(V, dtype=np.float32)
    np.add.at(imag, odd_indices, vals)
    y = np.empty(V, dtype=np.complex64)
    y.real = x
    y.imag = imag
    return y

